# revision 51
# baseline (speedup 1.0000x reference)
"""MLA (multi-head latent attention) Bass kernel for Trainium2, 8 NeuronCores.

Sharding: core i handles batch b = i // 2 and head-group g = i % 2
(8 of the 16 heads).  Each core computes a partial output (its heads'
contribution through out_proj); the host sums the two partials per batch
and adds a constant row (b_kvu_v @ w_o + b_o), which is exact because
softmax rows sum to 1 so the V-bias passes through attention additively.

All matmul operands are bf16 (1 cycle/row on the PE regardless of
output width); accumulation stays f32 in PSUM.  No PE transposes: both
x -> xT and ctx -> ctxT go through the DMA XBAR (dma_start_transpose,
2-byte dtypes) after an f32->bf16 rounding copy on GpSimd/DVE.

Structure (single TileContext; the Tile list-scheduler dispatches ready
instructions by emission-order priority, so emission IS the schedule):
  piece(p), p=0..3 (512 tokens each): latents kv_latT/q_latT{0,1}
    [128,S] = W^T xT (+bias, DVE); KT/QT [128, 4 chunks * S];
    V [128, NT*520] (64 cols/head + a ones col for the softmax denom).
  attention unit = (head pair hp, 512 queries): per key-chunk k one
    merged scores tile [128, 2 x fd] (both heads, disjoint 64-row PE
    groups) and ONE exp on ScalarE over a strided [128, 2, fd] AP
    (halves ScalarE's fixed per-call cost); causal diagonal via
    affine_select on Pool; PV re-uses exp tiles as stationary:
    ctx_psum[s-chunk] [128 queries, 65] accumulates over k, each
    s-chunk as one contiguous accumulation group (PSUM banks allow
    only one open group); per-partition reciprocal + scalar multiply
    (DVE) normalize into a token-major pair tile, DMA-transposed into
    ctxT.  Trailing PV groups/retires are deferred into the NEXT
    unit's slots ("pending") and projection pieces 2/3 + out_proj
    chunks are drained one sub-step per slot ("bg"), because the
    4-deep PE wait queue blocks later ready instructions behind
    waiting ones - bursts would starve ScalarE.
  out_proj per 128-token chunk: 4x128-contraction accumulate into
  [128,512] PSUM halves, DVE copy, DMA out; the last 4 chunks are
  gated per-128-column ctxT transposes of the final unit.
PSUM budget: scores 2x2 banks + ctx 2x1 + piece/out_proj ring 2 = 8.
"""

import numpy as np

import concourse.bass as bass
import concourse.bacc as bacc
import concourse.mybir as mybir
import concourse.tile as tile

DIM = 1024
NUM_HEADS = 16
HEAD_DIM = 64
LAT = 128
QR = 256
B = 4
NCORES = 8
ND = DIM // 128       # 8 d-chunks
NHL = 8               # heads per core
F32 = mybir.dt.float32
BF16 = mybir.dt.bfloat16
FP8 = mybir.dt.float8e4
AF = mybir.ActivationFunctionType
DR = mybir.MatmulPerfMode.DoubleRow

# fp8 is used ONLY on the Q-path (q_lat and QT projections): softmax squashes
# the ~2% fp8 rms error to ~0.1% on probs (scores err * 0.125 * score scale).
# The V/ctx/out path keeps bf16 -- fp8 there passes its full error to the
# output.  w_qc/w_qu are host-scaled by 64 into fp8-normal range; the
# projection epilogues descale by 1/64.
WU_SCALE = 64.0


def _pieces(total, w=512):
    return [(o, min(w, total - o)) for o in range(0, total, w)]


def build_mla(S=2048, mmdt=BF16):
    """Build the per-core Bass program (same SPMD program on all 8 cores)."""
    assert S % 512 == 0
    SH = S // 2           # s-half width
    NT = S // 128         # number of 128-token chunks
    NP = S // 512         # number of 512-token pieces

    nc = bacc.Bacc()

    # x and weights arrive host-rounded (bf16 / scaled fp8) in SBUF layout:
    # halves DMA bytes and removes all on-device staging/rounding copies.
    x_d = nc.declare_dram_parameter("x", [S, DIM], BF16, isOutput=False)
    w_kvc_d = nc.declare_dram_parameter("w_kvc", [128, ND * LAT], BF16,
                                        isOutput=False)
    w_qc_d = nc.declare_dram_parameter("w_qc", [128, ND * QR], FP8,
                                       isOutput=False)
    w_kvu_k_d = nc.declare_dram_parameter("w_kvu_k", [128, 512], BF16,
                                          isOutput=False)
    w_kvu_v_d = nc.declare_dram_parameter("w_kvu_v", [128, 512], BF16,
                                          isOutput=False)
    w_qu_d = nc.declare_dram_parameter("w_qu", [128, 1024], FP8,
                                       isOutput=False)
    w_o_d = nc.declare_dram_parameter("w_o", [128, 4 * DIM], BF16,
                                      isOutput=False)
    # b_kvc / b_qc are folded on the host into effective K/Q up-proj biases
    # (and the V-bias into the host const row), so the latent epilogues are
    # pure copies/scales: b_all = [b_qu_eff (4 chunks) | b_k_eff (4 chunks)]
    b_all_d = nc.declare_dram_parameter("b_all", [128, 8], F32, isOutput=False)
    out_d = nc.declare_dram_parameter("out", [S, DIM], F32, isOutput=True)

    with tile.TileContext(nc) as tc:
        with (
            tc.tile_pool(name="wts", bufs=1) as wts,
            tc.tile_pool(name="big", bufs=1) as big,
            tc.tile_pool(name="xbp", bufs=3) as xbp,
            tc.tile_pool(name="attn", bufs=1) as attn,
            tc.tile_pool(name="cpp", bufs=2) as cpp,
            tc.tile_pool(name="obp", bufs=2) as obp,
            tc.tile_pool(name="scps", bufs=1, space="PSUM") as scps,
            tc.tile_pool(name="ctxps", bufs=1, space="PSUM") as ctxps,
        ):
            # ---- persistent products -----------------------------------
            # xT is split per 512-token piece: the dependency tracker is
            # conservative across one big tile, so a single xT would make
            # later transposes wait on earlier pieces' matmul reads.
            xTp = [big.tile([128, ND * 512], mmdt, name=f"xT{j}")
                   for j in range(NP)]
            xTp_v = [t[:].rearrange("p (d t) -> p d t", d=ND) for t in xTp]
            # fp8 shadow of xT for the q_lat DoubleRow projection
            xT8p = [big.tile([128, ND * 512], FP8, name=f"xT8{j}")
                    for j in range(NP)]
            xT8p_v = [t[:].rearrange("p (d t) -> p d t", d=ND) for t in xT8p]
            kv_latT = big.tile([128, S], mmdt, name="kv_latT")
            # q_latT halves adjacent in one fp8 tile so QT can contract both
            # 128-blocks of QR in a single DoubleRow matmul
            q_latT = big.tile([128, 2 * S], FP8, name="q_latT")
            q_latT_v = q_latT[:].rearrange("p (g t) -> p g t", g=2)
            KT = big.tile([128, 4 * S], mmdt, name="KT")
            QT = big.tile([128, 4 * S], mmdt, name="QT")
            V = big.tile([128, NT * 520], mmdt, name="V")
            v_view = V[:].rearrange("p (k h c) -> p k h c", h=NHL, c=65)
            ctxT = big.tile([128, 4 * S], mmdt, name="ctxT")
            ctxT_v = ctxT[:].rearrange("p (c t) -> p c t", c=4)

            # ones columns of V (col 64 of each 65-wide head block)
            nc.gpsimd.memset(v_view[:, :, :, 64:65], 1.0)

            # ---- weights into SBUF (direct DMA, host-rounded) ----------
            w_kvc_sb = wts.tile([128, DIM], mmdt, name="w_kvc_sb")
            w_qc_sb = wts.tile([128, ND * QR], FP8, name="w_qc_sb")
            w_qc8_v = w_qc_sb[:].rearrange("p (d q) -> p d q", d=ND)
            w_kvu_k_sb = wts.tile([128, 512], mmdt, name="w_kvu_k_sb")
            w_kvu_v_sb = wts.tile([128, 512], mmdt, name="w_kvu_v_sb")
            w_qu_sb = wts.tile([128, 1024], FP8, name="w_qu_sb")
            w_qu8_v = w_qu_sb[:].rearrange("p (g q) -> p g q", g=2)
            w_o_sb = wts.tile([128, 4 * DIM], mmdt, name="w_o_sb")
            b_all_sb = wts.tile([128, 8], F32, name="b_all_sb")
            b_qu_sb = b_all_sb[:, 0:4]
            b_kvu_k_sb = b_all_sb[:, 4:8]

            def wload_o():
                # four DMAs, not one: a single 3.2us transfer head-of-line
                # blocks the shared DMA device at awkward times
                for cc in range(4):
                    nc.sync.dma_start(
                        out=w_o_sb[:, DIM * cc:DIM * cc + DIM],
                        in_=w_o_d[:, DIM * cc:DIM * cc + DIM])

            # ---- emission helpers --------------------------------------
            def x_chunk(q, warm=False):
                j, lq = q // 4, q % 4
                xb = xbp.tile([128, DIM], mmdt, tag="xb", bufs=3)
                nc.sync.dma_start(
                    out=xb[:], in_=x_d[128 * q:128 * q + 128, :])
                nc.sync.dma_start_transpose(
                    xTp_v[j][:, :, 128 * lq:128 * lq + 128], xb[:])
                nc.gpsimd.tensor_copy(
                    xT8p_v[j][:, :, 128 * lq:128 * lq + 128],
                    xTp_v[j][:, :, 128 * lq:128 * lq + 128])
                if warm:
                    # tiny matmuls keep the PE p-state ramp alive before the
                    # first projection burst
                    wps = scps.tile([128, 1024], F32, tag="sc", bufs=2,
                                    name=f"warm_{q}")
                    for i in range(8):
                        nc.tensor.matmul(wps[0:1, 0:1], xb[0:1, 0:1],
                                         xb[0:1, 0:1], start=True, stop=True)

            def x_stage(p):
                """Load, round, and DMA-transpose x tokens [512p, 512p+512)."""
                for q in range(4 * p, 4 * p + 4):
                    x_chunk(q)

            def piece(pj, o, width=512, to_bg=False, es=False):
                """All projections for tokens [o, o+width).

                With to_bg=True the sub-steps are queued on `bg` and drained
                one per attention slot, so they fill engine-idle time instead
                of preempting the next unit's QK matmuls.
                """
                p = o
                items = []

                xj, xo = o // 512, o % 512

                def lat_epi(dst_ap, acc_ap, scale):
                    """Latent epilogue: pure copy/scale (biases host-folded).
                    ScalarE (es=True) offloads the startup piece's epilogues
                    off DVE's serial queue while exp hasn't started yet."""
                    if es:
                        nc.scalar.activation(dst_ap, acc_ap, AF.Copy,
                                             scale=scale)
                    elif scale == 1.0:
                        nc.vector.tensor_copy(dst_ap, acc_ap)
                    else:
                        nc.vector.tensor_scalar_mul(dst_ap, acc_ap, scale)

                def _lat_kv():
                    state = {}

                    def part(d0, d1):
                        def emit():
                            if 'acc' not in state:
                                state['acc'] = pj.tile(
                                    [128, 512], F32, tag="pj", bufs=2,
                                    name=f"pjkv_{p}")
                            acc = state['acc']
                            for dc in range(d0, d1):
                                nc.tensor.matmul(
                                    acc[:, :width],
                                    w_kvc_sb[:, 128 * dc:128 * dc + 128],
                                    xTp_v[xj][:, dc, xo:xo + width],
                                    start=(dc == 0), stop=(dc == ND - 1))
                            if d1 == ND:
                                lat_epi(kv_latT[:, o:o + width],
                                        acc[:, :width], 1.0)
                        return emit
                    return [part(0, 3), part(3, 6), part(6, 8)]

                def _lat_q(coloff, dsto):
                    """q_lat half: DoubleRow fp8 over d-chunk pairs."""
                    state = {}

                    def part(p0, p1):
                        def emit():
                            if 'acc' not in state:
                                state['acc'] = pj.tile(
                                    [128, 512], F32, tag="pj", bufs=2,
                                    name=f"pjq_{p}_{coloff}")
                            acc = state['acc']
                            for dp in range(p0, p1):
                                nc.tensor.matmul(
                                    acc[:, :width],
                                    w_qc8_v[:, 2 * dp:2 * dp + 2,
                                            coloff:coloff + 128],
                                    xT8p_v[xj][:, 2 * dp:2 * dp + 2,
                                               xo:xo + width],
                                    start=(dp == 0), stop=(dp == ND // 2 - 1),
                                    perf_mode=DR)
                            if p1 == ND // 2:
                                lat_epi(q_latT[:, dsto + o:dsto + o + width],
                                        acc[:, :width], 1.0 / WU_SCALE)
                        return emit
                    return [part(0, 2), part(2, 4)]

                items.extend(_lat_kv())
                items.extend(_lat_q(0, 0))
                items.extend(_lat_q(128, S))

                def _qt(c):
                    def emit():
                        qp2 = pj.tile([128, 512], F32, tag="pj", bufs=2,
                                      name=f"pjq_{p}_{c}")
                        nc.tensor.matmul(
                            qp2[:, :width],
                            w_qu8_v[:, :, 128 * c:128 * c + 128],
                            q_latT_v[:, :, o:o + width],
                            start=True, stop=True, perf_mode=DR)
                        nc.vector.tensor_scalar(
                            QT[:, c * S + o:c * S + o + width], qp2[:, :width],
                            1.0 / WU_SCALE, b_qu_sb[:, c:c + 1],
                            op0=mybir.AluOpType.mult, op1=mybir.AluOpType.add)
                    return emit

                def _kt(c):
                    def emit():
                        kp = pj.tile([128, 512], F32, tag="pj", bufs=2,
                                     name=f"pjk_{p}_{c}")
                        nc.tensor.matmul(
                            kp[:, :width], w_kvu_k_sb[:, 128 * c:128 * c + 128],
                            kv_latT[:, o:o + width], start=True, stop=True)
                        nc.vector.tensor_scalar_add(
                            KT[:, c * S + o:c * S + o + width], kp[:, :width],
                            b_kvu_k_sb[:, c:c + 1])
                    return emit

                for c in range(4):
                    items.append(_qt(c))
                    items.append(_kt(c))

                def _v(q):
                    def emit():
                        vp = pj.tile([128, 512], F32, tag="pj", bufs=2,
                                     name=f"pjv_{q}")
                        nc.tensor.matmul(vp[:], kv_latT[:, 128 * q:128 * q + 128],
                                         w_kvu_v_sb[:], start=True, stop=True)
                        nc.vector.tensor_copy(
                            v_view[:, q, :, 0:64],
                            vp[:].rearrange("p (h c) -> p h c", c=64))
                    return emit

                for q in range(o // 128, (o + width) // 128):
                    items.append(_v(q))
                if to_bg:
                    bg.extend(items)
                else:
                    for it in items:
                        it()

            pending = []  # deferred closures, drained into later QK/exp slots
            bg = []       # background closures (pieces, out_proj), 1 per slot

            def drain(n):
                for _ in range(min(n, len(pending))):
                    pending.pop(0)()

            def drain_bg(n):
                for _ in range(min(n, len(bg))):
                    bg.pop(0)()

            def attn_unit(hp, s0, SW=512, last=False, bg_n=1,
                          on_chunk=None):
                """Attention for queries [s0, s0+SW), both heads of pair hp.

                One merged exp per key-chunk covers both heads ([128, 2, fd]
                strided AP over a shared scores tile).  PV accumulation
                groups are emitted contiguously per (head, s-chunk) -- PSUM
                banks support only one open group at a time -- lagged into
                the QK/exp slots; trailing groups, the retires, and the
                ctxT transposes are deferred into the NEXT unit's slots via
                `pending` (the 4-deep PE wait queue would otherwise block
                the next unit's QK behind not-yet-ready PV matmuls).
                """
                base = s0 // 128
                nch = SW // 128
                kmax = base + nch
                cp = cpp.tile([128, 512], mmdt, tag="cp", name=f"cp_{hp}_{s0}")
                ctxs = [ctxps.tile([128, 512], F32, tag="ctx", bufs=2,
                                   name=f"ctx_{hp}_{s0}_{i}")
                        for i in range(2)]
                exs = []

                def pv_group(h2, c):
                    h = 2 * hp + h2
                    ctx = ctxs[h2]
                    klast = base + c

                    def emit():
                        for k in range(klast + 1):
                            rel = max(s0, 128 * k) - s0
                            cs = 512 * h2 + 128 * c - rel
                            nc.tensor.matmul(
                                ctx[:, 128 * c:128 * c + 65],
                                exs[k][:, cs:cs + 128],
                                V[:, 520 * k + 65 * h:520 * k + 65 * h + 65],
                                start=(k == 0), stop=(k == klast))
                        if last:
                            rec = attn.tile([128, 1], F32, tag="rec1", bufs=4,
                                            name=f"rec1_{hp}_{s0}_{h2}_{c}")
                            nc.vector.reciprocal(
                                rec[:], ctx[:, 128 * c + 64:128 * c + 65])
                            nc.vector.tensor_scalar_mul(
                                cp[:, 128 * c + 64 * h2:128 * c + 64 * h2 + 64],
                                ctx[:, 128 * c:128 * c + 64], rec[:, 0:1])
                            if h2 == 1:
                                nc.sync.dma_start_transpose(
                                    ctxT_v[:, hp, s0 + 128 * c:s0 + 128 * c + 128]
                                    .rearrange("p (b t) -> p b t", t=128),
                                    cp[:, 128 * c:128 * c + 128])
                                if on_chunk is not None:
                                    on_chunk(c)
                    return emit

                def retire(h2):
                    ctx = ctxs[h2]

                    def emit():
                        rec = attn.tile([128, 4], F32, tag="rec", bufs=4,
                                        name=f"rec_{hp}_{s0}_{h2}")
                        nc.vector.reciprocal(
                            rec[:, :nch],
                            ctx[:].rearrange("p (c u) -> p c u", u=128)[:, :nch, 64])
                        for c in range(nch):
                            nc.vector.tensor_scalar_mul(
                                cp[:, 128 * c + 64 * h2:128 * c + 64 * h2 + 64],
                                ctx[:, 128 * c:128 * c + 64], rec[:, c:c + 1])
                    return emit

                def tp():
                    def emit():
                        nc.sync.dma_start_transpose(
                            ctxT_v[:, hp, s0:s0 + SW].rearrange(
                                "p (b t) -> p b t", t=128),
                            cp[:, :SW])
                    return emit

                lag = 2 if s0 == 0 else 0
                inslot = nch if last else max(1, nch - 2)
                emitted = 0
                for k in range(kmax):
                    t0 = 128 * k
                    ss = max(s0, t0)
                    fd = s0 + SW - ss
                    sc = scps.tile([128, 1024], F32, tag="sc", bufs=2,
                                   name=f"sc_{hp}_{s0}_{k}")
                    for h2 in range(2):
                        po = 64 * h2
                        nc.tensor.matmul(
                            sc[:, 512 * h2:512 * h2 + fd],
                            KT[po:po + 64, hp * S + t0:hp * S + t0 + 128],
                            QT[po:po + 64, hp * S + ss:hp * S + ss + fd],
                            start=True, stop=True)
                    ex = attn.tile([128, 1024], mmdt, tag="ex", bufs=21,
                                   name=f"ex_{hp}_{s0}_{k}")
                    exs.append(ex)
                    sc3 = sc[:].rearrange("p (g q) -> p g q", g=2)[:, :, :fd]
                    ex3 = ex[:].rearrange("p (g q) -> p g q", g=2)[:, :, :fd]
                    nc.scalar.activation(ex3, sc3, AF.Exp, scale=0.125)
                    if t0 >= s0:
                        nc.gpsimd.affine_select(
                            out=ex[:].rearrange("p (g q) -> p g q", g=2)[:, :, 0:128],
                            in_=ex[:].rearrange("p (g q) -> p g q", g=2)[:, :, 0:128],
                            pattern=[[0, 2], [1, 128]],
                            compare_op=mybir.AluOpType.is_ge,
                            fill=0.0, base=0, channel_multiplier=-1)
                    drain(2)
                    drain_bg(bg_n)
                    c = k - base - lag
                    if 0 <= c < inslot:
                        pv_group(0, c)()
                        pv_group(1, c)()
                        emitted = c + 1
                for c in range(emitted, nch):
                    if last or nch <= 2:
                        pv_group(0, c)()
                        pv_group(1, c)()
                    else:
                        pending.append(pv_group(0, c))
                        pending.append(pv_group(1, c))
                if not last:
                    pending.append(retire(0))
                    pending.append(retire(1))
                    pending.append(tp())

            def out_chunk(ops, si, to_bg=False, to_pending=False):
                """out_proj for tokens [128si, 128si+128)."""
                ob = obp.tile([128, DIM], F32, tag="ob", name=f"ob_{si}")

                def _half(u):
                    def emit():
                        op = ops.tile([128, 512], F32, tag="pj", bufs=2,
                                      name=f"op_{si}_{u}")
                        for cc in range(4):
                            nc.tensor.matmul(
                                op[:],
                                ctxT_v[:, cc, 128 * si:128 * si + 128],
                                w_o_sb[:, DIM * cc + 512 * u:DIM * cc + 512 * u + 512],
                                start=(cc == 0), stop=(cc == 3))
                        nc.vector.tensor_copy(ob[:, 512 * u:512 * u + 512], op[:])
                        if u == 1:
                            nc.sync.dma_start(
                                out=out_d[128 * si:128 * si + 128, :], in_=ob[:])
                    return emit

                if to_bg:
                    bg.append(_half(0))
                    bg.append(_half(1))
                elif to_pending:
                    pending.append(_half(0))
                    pending.append(_half(1))
                else:
                    _half(0)()
                    _half(1)()

            def tail_partial(ops, si):
                """out_proj cc=0..2 partial for tokens [128si, +128) -> SBUF.

                Runs inside the last unit's slots (hp<=2 ctxT ready); only
                the cc=3 matmul + add remains after the final transposes.
                """
                pt = obp.tile([128, DIM], mmdt, tag="pt", bufs=4,
                              name=f"pt_{si}")

                def _half(u):
                    def emit():
                        op = ops.tile([128, 512], F32, tag="pj", bufs=2,
                                      name=f"ptp_{si}_{u}")
                        for cc in range(3):
                            nc.tensor.matmul(
                                op[:],
                                ctxT_v[:, cc, 128 * si:128 * si + 128],
                                w_o_sb[:, DIM * cc + 512 * u:DIM * cc + 512 * u + 512],
                                start=(cc == 0), stop=(cc == 2))
                        nc.vector.tensor_copy(pt[:, 512 * u:512 * u + 512],
                                              op[:])
                    return emit
                bg.append(_half(0))
                bg.append(_half(1))
                return pt

            def tail_finish(ops, si, pt):
                """out_proj cc=3 + partial add + store, via pending."""
                ob = obp.tile([128, DIM], F32, tag="ob", name=f"ob_{si}")

                def _half(u):
                    def emit():
                        op = ops.tile([128, 512], F32, tag="pj", bufs=2,
                                      name=f"tf_{si}_{u}")
                        nc.tensor.matmul(
                            op[:],
                            ctxT_v[:, 3, 128 * si:128 * si + 128],
                            w_o_sb[:, 3 * DIM + 512 * u:3 * DIM + 512 * u + 512],
                            start=True, stop=True)
                        nc.vector.tensor_tensor(
                            ob[:, 512 * u:512 * u + 512], op[:],
                            pt[:, 512 * u:512 * u + 512],
                            op=mybir.AluOpType.add)
                        if u == 1:
                            nc.sync.dma_start(
                                out=out_d[128 * si:128 * si + 128, :],
                                in_=ob[:])
                    return emit
                pending.append(_half(0))
                pending.append(_half(1))

            # ---- emission schedule -------------------------------------
            # Startup DMAs ordered by first use on the exp critical path
            # (kv_lat -> q_lat -> KT/QT -> first QK); the cost model
            # serializes all DMA transfers on one shared device, so queue
            # order IS arrival order.
            with tc.tile_pool(name="pjps", bufs=1, space="PSUM") as pj:
                x_chunk(0, warm=True)
                x_chunk(1, warm=True)
                nc.sync.dma_start(out=w_kvc_sb[:], in_=w_kvc_d[:, :])
                x_chunk(2, warm=True)
                x_chunk(3, warm=True)
                nc.sync.dma_start(out=w_qc_sb[:], in_=w_qc_d[:, :])
                nc.sync.dma_start(out=w_kvu_k_sb[:], in_=w_kvu_k_d[:, :])
                nc.sync.dma_start(out=w_qu_sb[:], in_=w_qu_d[:, :])
                nc.sync.dma_start(out=b_all_sb[:], in_=b_all_d[:, :])
                nc.sync.dma_start(out=w_kvu_v_sb[:], in_=w_kvu_v_d[:, :])
                x_chunk(4)
                x_chunk(5)
                x_chunk(6)
                x_chunk(7)
                piece(pj, 0, 512, es=True)
                # pieces for tokens 512-1023 fill s-block 0's idle slots;
                # split 256/256 so the first items only touch xT chunks 4-5
                # (landed by the time block-0's early slots drain them)
                piece(pj, 512, 256, to_bg=True)
                piece(pj, 768, 256, to_bg=True)
                for hp in range(4):
                    attn_unit(hp, 0, bg_n=(1, 2, 4, 4)[hp])
                x_stage(2)
                piece(pj, 1024, 512, to_bg=True)
                for hp in range(4):
                    attn_unit(hp, 512)
                x_stage(3)
                drain_bg(len(bg))
                wload_o()
            with tc.tile_pool(name="ops", bufs=1, space="PSUM") as ops:
                piece(ops, 1536, 512, to_bg=True)
                for hp in range(4):
                    attn_unit(hp, 1024)
                    out_chunk(ops, hp, to_bg=True)
                for hp in range(3):
                    attn_unit(hp, 1536)
                    out_chunk(ops, 4 + 2 * hp, to_bg=True)
                    out_chunk(ops, 5 + 2 * hp, to_bg=True)
                    if hp == 2:
                        out_chunk(ops, 10, to_bg=True)
                        out_chunk(ops, 11, to_bg=True)
                pts = {si: tail_partial(ops, si) for si in range(12, NT)}
                attn_unit(3, 1536, last=True,
                          on_chunk=lambda c: tail_finish(ops, 12 + c,
                                                         pts[12 + c]))
                drain(len(pending))
                drain_bg(len(bg))

    nc.finalize()
    return nc


def shard_inputs(inputs, S=2048):
    """Build the 8 per-core input maps (host-rounded, SBUF-layout)."""
    np_bf16 = mybir.dt.np(BF16)
    np_fp8 = mybir.dt.np(FP8)
    f = lambda a: np.ascontiguousarray(np.asarray(a, dtype=np.float32))

    def sb_layout(w, dt):
        """[c*128, q] -> [128, c*q] (partition-major chunks), cast to dt."""
        c = w.shape[0] // 128
        return np.ascontiguousarray(
            w.reshape(c, 128, -1).transpose(1, 0, 2).reshape(128, -1)
        ).astype(dt)

    x = f(inputs["x"])
    w_kvc, b_kvc = f(inputs["w_kvc"]), f(inputs["b_kvc"])
    w_kvu, b_kvu = f(inputs["w_kvu"]), f(inputs["b_kvu"])
    w_qc, b_qc = f(inputs["w_qc"]), f(inputs["b_qc"])
    w_qu, b_qu = f(inputs["w_qu"]), f(inputs["b_qu"])
    w_o = f(inputs["w_o"])
    w_kvc_l = sb_layout(w_kvc, np_bf16)
    w_qc_l = sb_layout(w_qc * WU_SCALE, np_fp8)
    # fold the latent biases into the up-projection biases
    b_qu_eff = b_qc @ w_qu + b_qu          # [1024]
    b_k_eff = b_kvc @ w_kvu[:, :1024] + b_kvu[:1024]
    in_maps = []
    for core in range(NCORES):
        b = core // 2
        g = core % 2
        cs = slice(512 * g, 512 * g + 512)
        in_maps.append({
            "x": x[b].astype(np_bf16),
            "w_kvc": w_kvc_l,
            "w_qc": w_qc_l,
            "w_kvu_k": w_kvu[:, 512 * g:512 * g + 512].astype(np_bf16),
            "w_kvu_v": w_kvu[:, 1024 + 512 * g:1024 + 512 * g + 512].astype(np_bf16),
            "w_qu": sb_layout(w_qu[:, cs] * WU_SCALE, np_fp8),
            "w_o": sb_layout(w_o[cs, :], np_bf16),
            "b_all": np.ascontiguousarray(np.concatenate([
                b_qu_eff[cs].reshape(4, 128).T,
                b_k_eff[cs].reshape(4, 128).T,
            ], axis=1)),
        })
    return in_maps


def gather_out(results, inputs, S=2048):
    """Sum the two per-batch partials and add the constant bias row."""
    f = lambda a: np.asarray(a, dtype=np.float32)
    # effective V bias incl. the folded b_kvc contribution
    b_v = f(inputs["b_kvc"]) @ f(inputs["w_kvu"])[:, DIM:] + f(inputs["b_kvu"])[DIM:]
    const_row = b_v @ f(inputs["w_o"]) + f(inputs["b_o"])
    out = np.empty((B, S, DIM), dtype=np.float32)
    for b in range(B):
        out[b] = results[2 * b]["out"] + results[2 * b + 1]["out"] + const_row
    return out


def kernel(**inputs) -> np.ndarray:
    from concourse.bass_utils import run_bass_kernel_spmd

    x = np.asarray(inputs["x"])
    S = x.shape[1]
    nc = build_mla(S=S)
    in_maps = shard_inputs(inputs, S=S)
    res = run_bass_kernel_spmd(nc, in_maps, list(range(NCORES))).results
    return gather_out(res, inputs, S=S)



# revision 55
# speedup vs baseline: 1.0029x; 1.0029x over previous
"""MLA (multi-head latent attention) Bass kernel for Trainium2, 8 NeuronCores.

Sharding: core i handles batch b = i // 2 and head-group g = i % 2
(8 of the 16 heads).  Each core computes a partial output (its heads'
contribution through out_proj); the host sums the two partials per batch
and adds a constant row (b_kvu_v @ w_o + b_o), which is exact because
softmax rows sum to 1 so the V-bias passes through attention additively.

All matmul operands are bf16 (1 cycle/row on the PE regardless of
output width); accumulation stays f32 in PSUM.  No PE transposes: both
x -> xT and ctx -> ctxT go through the DMA XBAR (dma_start_transpose,
2-byte dtypes) after an f32->bf16 rounding copy on GpSimd/DVE.

Structure (single TileContext; the Tile list-scheduler dispatches ready
instructions by emission-order priority, so emission IS the schedule):
  piece(p), p=0..3 (512 tokens each): latents kv_latT/q_latT{0,1}
    [128,S] = W^T xT (+bias, DVE); KT/QT [128, 4 chunks * S];
    V [128, NT*520] (64 cols/head + a ones col for the softmax denom).
  attention unit = (head pair hp, 512 queries): per key-chunk k one
    merged scores tile [128, 2 x fd] (both heads, disjoint 64-row PE
    groups) and ONE exp on ScalarE over a strided [128, 2, fd] AP
    (halves ScalarE's fixed per-call cost); causal diagonal via
    affine_select on Pool; PV re-uses exp tiles as stationary:
    ctx_psum[s-chunk] [128 queries, 65] accumulates over k, each
    s-chunk as one contiguous accumulation group (PSUM banks allow
    only one open group); per-partition reciprocal + scalar multiply
    (DVE) normalize into a token-major pair tile, DMA-transposed into
    ctxT.  Trailing PV groups/retires are deferred into the NEXT
    unit's slots ("pending") and projection pieces 2/3 + out_proj
    chunks are drained one sub-step per slot ("bg"), because the
    4-deep PE wait queue blocks later ready instructions behind
    waiting ones - bursts would starve ScalarE.
  out_proj per 128-token chunk: 4x128-contraction accumulate into
  [128,512] PSUM halves, DVE copy, DMA out; the last 4 chunks are
  gated per-128-column ctxT transposes of the final unit.
PSUM budget: scores 2x2 banks + ctx 2x1 + piece/out_proj ring 2 = 8.
"""

import numpy as np

import concourse.bass as bass
import concourse.bacc as bacc
import concourse.mybir as mybir
import concourse.tile as tile

DIM = 1024
NUM_HEADS = 16
HEAD_DIM = 64
LAT = 128
QR = 256
B = 4
NCORES = 8
ND = DIM // 128       # 8 d-chunks
NHL = 8               # heads per core
F32 = mybir.dt.float32
BF16 = mybir.dt.bfloat16
FP8 = mybir.dt.float8e4
AF = mybir.ActivationFunctionType
DR = mybir.MatmulPerfMode.DoubleRow

# fp8 is used ONLY on the Q-path (q_lat and QT projections): softmax squashes
# the ~2% fp8 rms error to ~0.1% on probs (scores err * 0.125 * score scale).
# The V/ctx/out path keeps bf16 -- fp8 there passes its full error to the
# output.  w_qc/w_qu are host-scaled by 64 into fp8-normal range; the
# projection epilogues descale by 1/64.
WU_SCALE = 64.0


def _pieces(total, w=512):
    return [(o, min(w, total - o)) for o in range(0, total, w)]


def build_mla(S=2048, mmdt=BF16):
    """Build the per-core Bass program (same SPMD program on all 8 cores)."""
    assert S % 512 == 0
    SH = S // 2           # s-half width
    NT = S // 128         # number of 128-token chunks
    NP = S // 512         # number of 512-token pieces

    nc = bacc.Bacc()

    # x and weights arrive host-rounded (bf16 / scaled fp8) in SBUF layout:
    # halves DMA bytes and removes all on-device staging/rounding copies.
    x_d = nc.declare_dram_parameter("x", [S, DIM], BF16, isOutput=False)
    w_kvc_d = nc.declare_dram_parameter("w_kvc", [128, ND * LAT], BF16,
                                        isOutput=False)
    w_qc_d = nc.declare_dram_parameter("w_qc", [128, ND * QR], FP8,
                                       isOutput=False)
    w_kvu_k_d = nc.declare_dram_parameter("w_kvu_k", [128, 512], BF16,
                                          isOutput=False)
    w_kvu_v_d = nc.declare_dram_parameter("w_kvu_v", [128, 512], BF16,
                                          isOutput=False)
    w_qu_d = nc.declare_dram_parameter("w_qu", [128, 1024], FP8,
                                       isOutput=False)
    w_o_d = nc.declare_dram_parameter("w_o", [128, 4 * DIM], BF16,
                                      isOutput=False)
    # b_kvc / b_qc are folded on the host into effective K/Q up-proj biases
    # (and the V-bias into the host const row), so the latent epilogues are
    # pure copies/scales: b_all = [b_qu_eff (4 chunks) | b_k_eff (4 chunks)]
    b_all_d = nc.declare_dram_parameter("b_all", [128, 8], F32, isOutput=False)
    out_d = nc.declare_dram_parameter("out", [S, DIM], F32, isOutput=True)

    with tile.TileContext(nc) as tc:
        with (
            tc.tile_pool(name="wts", bufs=1) as wts,
            tc.tile_pool(name="big", bufs=1) as big,
            tc.tile_pool(name="xbp", bufs=3) as xbp,
            tc.tile_pool(name="attn", bufs=1) as attn,
            tc.tile_pool(name="cpp", bufs=2) as cpp,
            tc.tile_pool(name="obp", bufs=2) as obp,
            tc.tile_pool(name="scps", bufs=1, space="PSUM") as scps,
            tc.tile_pool(name="ctxps", bufs=1, space="PSUM") as ctxps,
        ):
            # ---- persistent products -----------------------------------
            # xT is split per 512-token piece: the dependency tracker is
            # conservative across one big tile, so a single xT would make
            # later transposes wait on earlier pieces' matmul reads.
            xTp = [big.tile([128, ND * 512], mmdt, name=f"xT{j}")
                   for j in range(NP)]
            xTp_v = [t[:].rearrange("p (d t) -> p d t", d=ND) for t in xTp]
            # fp8 shadow of xT for the q_lat DoubleRow projection
            xT8p = [big.tile([128, ND * 512], FP8, name=f"xT8{j}")
                    for j in range(NP)]
            xT8p_v = [t[:].rearrange("p (d t) -> p d t", d=ND) for t in xT8p]
            kv_latT = big.tile([128, S], mmdt, name="kv_latT")
            # q_latT halves adjacent in one fp8 tile so QT can contract both
            # 128-blocks of QR in a single DoubleRow matmul
            q_latT = big.tile([128, 2 * S], FP8, name="q_latT")
            q_latT_v = q_latT[:].rearrange("p (g t) -> p g t", g=2)
            KT = big.tile([128, 4 * S], mmdt, name="KT")
            QT = big.tile([128, 4 * S], mmdt, name="QT")
            V = big.tile([128, NT * 520], mmdt, name="V")
            v_view = V[:].rearrange("p (k h c) -> p k h c", h=NHL, c=65)
            ctxT = big.tile([128, 4 * S], mmdt, name="ctxT")
            ctxT_v = ctxT[:].rearrange("p (c t) -> p c t", c=4)

            # ones columns of V (col 64 of each 65-wide head block)
            nc.gpsimd.memset(v_view[:, :, :, 64:65], 1.0)

            # ---- weights into SBUF (direct DMA, host-rounded) ----------
            w_kvc_sb = wts.tile([128, DIM], mmdt, name="w_kvc_sb")
            w_qc_sb = wts.tile([128, ND * QR], FP8, name="w_qc_sb")
            w_qc8_v = w_qc_sb[:].rearrange("p (d q) -> p d q", d=ND)
            w_kvu_k_sb = wts.tile([128, 512], mmdt, name="w_kvu_k_sb")
            w_kvu_v_sb = wts.tile([128, 512], mmdt, name="w_kvu_v_sb")
            w_qu_sb = wts.tile([128, 1024], FP8, name="w_qu_sb")
            w_qu8_v = w_qu_sb[:].rearrange("p (g q) -> p g q", g=2)
            w_o_sb = wts.tile([128, 4 * DIM], mmdt, name="w_o_sb")
            b_all_sb = wts.tile([128, 8], F32, name="b_all_sb")
            b_qu_sb = b_all_sb[:, 0:4]
            b_kvu_k_sb = b_all_sb[:, 4:8]

            def wload_o():
                # four DMAs, not one: a single 3.2us transfer head-of-line
                # blocks the shared DMA device at awkward times
                for cc in range(4):
                    nc.sync.dma_start(
                        out=w_o_sb[:, DIM * cc:DIM * cc + DIM],
                        in_=w_o_d[:, DIM * cc:DIM * cc + DIM])

            # ---- emission helpers --------------------------------------
            def x_chunk(q, warm=False):
                j, lq = q // 4, q % 4
                xb = xbp.tile([128, DIM], mmdt, tag="xb", bufs=3)
                nc.sync.dma_start(
                    out=xb[:], in_=x_d[128 * q:128 * q + 128, :])
                nc.sync.dma_start_transpose(
                    xTp_v[j][:, :, 128 * lq:128 * lq + 128], xb[:])
                nc.gpsimd.tensor_copy(
                    xT8p_v[j][:, :, 128 * lq:128 * lq + 128],
                    xTp_v[j][:, :, 128 * lq:128 * lq + 128])
                if warm:
                    # tiny matmuls keep the PE p-state ramp alive before the
                    # first projection burst
                    wps = scps.tile([128, 1024], F32, tag="sc", bufs=2,
                                    name=f"warm_{q}")
                    for i in range(8):
                        nc.tensor.matmul(wps[0:1, 0:1], xb[0:1, 0:1],
                                         xb[0:1, 0:1], start=True, stop=True)

            def x_stage(p):
                """Load, round, and DMA-transpose x tokens [512p, 512p+512)."""
                for q in range(4 * p, 4 * p + 4):
                    x_chunk(q)

            def piece(pj, o, width=512, to_bg=False, es=False):
                """All projections for tokens [o, o+width).

                With to_bg=True the sub-steps are queued on `bg` and drained
                one per attention slot, so they fill engine-idle time instead
                of preempting the next unit's QK matmuls.
                """
                p = o
                items = []

                xj, xo = o // 512, o % 512

                def lat_epi(dst_ap, acc_ap, scale):
                    """Latent epilogue: pure copy/scale (biases host-folded).
                    ScalarE (es=True) offloads the startup piece's epilogues
                    off DVE's serial queue while exp hasn't started yet."""
                    if es:
                        nc.scalar.activation(dst_ap, acc_ap, AF.Copy,
                                             scale=scale)
                    elif scale == 1.0:
                        nc.vector.tensor_copy(dst_ap, acc_ap)
                    else:
                        nc.vector.tensor_scalar_mul(dst_ap, acc_ap, scale)

                def _lat_kv():
                    state = {}

                    def part(d0, d1):
                        def emit():
                            if 'acc' not in state:
                                state['acc'] = pj.tile(
                                    [128, 512], F32, tag="pj", bufs=2,
                                    name=f"pjkv_{p}")
                            acc = state['acc']
                            for dc in range(d0, d1):
                                nc.tensor.matmul(
                                    acc[:, :width],
                                    w_kvc_sb[:, 128 * dc:128 * dc + 128],
                                    xTp_v[xj][:, dc, xo:xo + width],
                                    start=(dc == 0), stop=(dc == ND - 1))
                            if d1 == ND:
                                lat_epi(kv_latT[:, o:o + width],
                                        acc[:, :width], 1.0)
                        return emit
                    return [part(0, 3), part(3, 6), part(6, 8)]

                def _lat_q(coloff, dsto):
                    """q_lat half: DoubleRow fp8 over d-chunk pairs."""
                    state = {}

                    def part(p0, p1):
                        def emit():
                            if 'acc' not in state:
                                state['acc'] = pj.tile(
                                    [128, 512], F32, tag="pj", bufs=2,
                                    name=f"pjq_{p}_{coloff}")
                            acc = state['acc']
                            for dp in range(p0, p1):
                                nc.tensor.matmul(
                                    acc[:, :width],
                                    w_qc8_v[:, 2 * dp:2 * dp + 2,
                                            coloff:coloff + 128],
                                    xT8p_v[xj][:, 2 * dp:2 * dp + 2,
                                               xo:xo + width],
                                    start=(dp == 0), stop=(dp == ND // 2 - 1),
                                    perf_mode=DR)
                            if p1 == ND // 2:
                                lat_epi(q_latT[:, dsto + o:dsto + o + width],
                                        acc[:, :width], 1.0 / WU_SCALE)
                        return emit
                    return [part(0, 2), part(2, 4)]

                items.extend(_lat_kv())
                items.extend(_lat_q(0, 0))
                items.extend(_lat_q(128, S))

                def _qt(c):
                    def emit():
                        qp2 = pj.tile([128, 512], F32, tag="pj", bufs=2,
                                      name=f"pjq_{p}_{c}")
                        nc.tensor.matmul(
                            qp2[:, :width],
                            w_qu8_v[:, :, 128 * c:128 * c + 128],
                            q_latT_v[:, :, o:o + width],
                            start=True, stop=True, perf_mode=DR)
                        nc.vector.tensor_scalar(
                            QT[:, c * S + o:c * S + o + width], qp2[:, :width],
                            1.0 / WU_SCALE, b_qu_sb[:, c:c + 1],
                            op0=mybir.AluOpType.mult, op1=mybir.AluOpType.add)
                    return emit

                def _kt(c):
                    def emit():
                        kp = pj.tile([128, 512], F32, tag="pj", bufs=2,
                                     name=f"pjk_{p}_{c}")
                        nc.tensor.matmul(
                            kp[:, :width], w_kvu_k_sb[:, 128 * c:128 * c + 128],
                            kv_latT[:, o:o + width], start=True, stop=True)
                        nc.vector.tensor_scalar_add(
                            KT[:, c * S + o:c * S + o + width], kp[:, :width],
                            b_kvu_k_sb[:, c:c + 1])
                    return emit

                for c in range(4):
                    items.append(_qt(c))
                    items.append(_kt(c))

                def _v(q):
                    def emit():
                        vp = pj.tile([128, 512], F32, tag="pj", bufs=2,
                                     name=f"pjv_{q}")
                        nc.tensor.matmul(vp[:], kv_latT[:, 128 * q:128 * q + 128],
                                         w_kvu_v_sb[:], start=True, stop=True)
                        nc.vector.tensor_copy(
                            v_view[:, q, :, 0:64],
                            vp[:].rearrange("p (h c) -> p h c", c=64))
                    return emit

                for q in range(o // 128, (o + width) // 128):
                    items.append(_v(q))
                if to_bg:
                    bg.extend(items)
                else:
                    for it in items:
                        it()

            pending = []  # deferred closures, drained into later QK/exp slots
            bg = []       # background closures (pieces, out_proj), 1 per slot

            def drain(n):
                for _ in range(min(n, len(pending))):
                    pending.pop(0)()

            def drain_bg(n):
                for _ in range(min(n, len(bg))):
                    bg.pop(0)()

            def attn_unit(hp, s0, SW=512, last=False, bg_n=1, bg_delay=0,
                          on_chunk=None):
                """Attention for queries [s0, s0+SW), both heads of pair hp.

                One merged exp per key-chunk covers both heads ([128, 2, fd]
                strided AP over a shared scores tile).  PV accumulation
                groups are emitted contiguously per (head, s-chunk) -- PSUM
                banks support only one open group at a time -- lagged into
                the QK/exp slots; trailing groups, the retires, and the
                ctxT transposes are deferred into the NEXT unit's slots via
                `pending` (the 4-deep PE wait queue would otherwise block
                the next unit's QK behind not-yet-ready PV matmuls).
                """
                base = s0 // 128
                nch = SW // 128
                kmax = base + nch
                cp = cpp.tile([128, 512], mmdt, tag="cp", name=f"cp_{hp}_{s0}")
                ctxs = [ctxps.tile([128, 512], F32, tag="ctx", bufs=2,
                                   name=f"ctx_{hp}_{s0}_{i}")
                        for i in range(2)]
                exs = []

                def pv_group(h2, c):
                    h = 2 * hp + h2
                    ctx = ctxs[h2]
                    klast = base + c

                    def emit():
                        for k in range(klast + 1):
                            rel = max(s0, 128 * k) - s0
                            cs = 512 * h2 + 128 * c - rel
                            nc.tensor.matmul(
                                ctx[:, 128 * c:128 * c + 65],
                                exs[k][:, cs:cs + 128],
                                V[:, 520 * k + 65 * h:520 * k + 65 * h + 65],
                                start=(k == 0), stop=(k == klast))
                        if last:
                            rec = attn.tile([128, 1], F32, tag="rec1", bufs=4,
                                            name=f"rec1_{hp}_{s0}_{h2}_{c}")
                            nc.vector.reciprocal(
                                rec[:], ctx[:, 128 * c + 64:128 * c + 65])
                            nc.vector.tensor_scalar_mul(
                                cp[:, 128 * c + 64 * h2:128 * c + 64 * h2 + 64],
                                ctx[:, 128 * c:128 * c + 64], rec[:, 0:1])
                            if h2 == 1:
                                nc.sync.dma_start_transpose(
                                    ctxT_v[:, hp, s0 + 128 * c:s0 + 128 * c + 128]
                                    .rearrange("p (b t) -> p b t", t=128),
                                    cp[:, 128 * c:128 * c + 128])
                                if on_chunk is not None:
                                    on_chunk(c)
                    return emit

                def retire(h2):
                    ctx = ctxs[h2]

                    def emit():
                        rec = attn.tile([128, 4], F32, tag="rec", bufs=4,
                                        name=f"rec_{hp}_{s0}_{h2}")
                        nc.vector.reciprocal(
                            rec[:, :nch],
                            ctx[:].rearrange("p (c u) -> p c u", u=128)[:, :nch, 64])
                        for c in range(nch):
                            nc.vector.tensor_scalar_mul(
                                cp[:, 128 * c + 64 * h2:128 * c + 64 * h2 + 64],
                                ctx[:, 128 * c:128 * c + 64], rec[:, c:c + 1])
                    return emit

                def tp():
                    def emit():
                        nc.sync.dma_start_transpose(
                            ctxT_v[:, hp, s0:s0 + SW].rearrange(
                                "p (b t) -> p b t", t=128),
                            cp[:, :SW])
                    return emit

                lag = 2 if s0 == 0 else 0
                inslot = nch if last else max(1, nch - 2)
                emitted = 0
                for k in range(kmax):
                    t0 = 128 * k
                    ss = max(s0, t0)
                    fd = s0 + SW - ss
                    sc = scps.tile([128, 1024], F32, tag="sc", bufs=2,
                                   name=f"sc_{hp}_{s0}_{k}")
                    for h2 in range(2):
                        po = 64 * h2
                        nc.tensor.matmul(
                            sc[:, 512 * h2:512 * h2 + fd],
                            KT[po:po + 64, hp * S + t0:hp * S + t0 + 128],
                            QT[po:po + 64, hp * S + ss:hp * S + ss + fd],
                            start=True, stop=True)
                    ex = attn.tile([128, 1024], mmdt, tag="ex", bufs=21,
                                   name=f"ex_{hp}_{s0}_{k}")
                    exs.append(ex)
                    sc3 = sc[:].rearrange("p (g q) -> p g q", g=2)[:, :, :fd]
                    ex3 = ex[:].rearrange("p (g q) -> p g q", g=2)[:, :, :fd]
                    nc.scalar.activation(ex3, sc3, AF.Exp, scale=0.125)
                    if t0 >= s0:
                        nc.gpsimd.affine_select(
                            out=ex[:].rearrange("p (g q) -> p g q", g=2)[:, :, 0:128],
                            in_=ex[:].rearrange("p (g q) -> p g q", g=2)[:, :, 0:128],
                            pattern=[[0, 2], [1, 128]],
                            compare_op=mybir.AluOpType.is_ge,
                            fill=0.0, base=0, channel_multiplier=-1)
                    drain(2)
                    if k >= bg_delay:
                        drain_bg(bg_n)
                    c = k - base - lag
                    if 0 <= c < inslot:
                        pv_group(0, c)()
                        pv_group(1, c)()
                        emitted = c + 1
                for c in range(emitted, nch):
                    if last or nch <= 2:
                        pv_group(0, c)()
                        pv_group(1, c)()
                    else:
                        pending.append(pv_group(0, c))
                        pending.append(pv_group(1, c))
                if not last:
                    pending.append(retire(0))
                    pending.append(retire(1))
                    pending.append(tp())

            def out_chunk(ops, si, to_bg=False, to_pending=False):
                """out_proj for tokens [128si, 128si+128)."""
                ob = obp.tile([128, DIM], F32, tag="ob", name=f"ob_{si}")

                def _half(u):
                    def emit():
                        op = ops.tile([128, 512], F32, tag="pj", bufs=2,
                                      name=f"op_{si}_{u}")
                        for cc in range(4):
                            nc.tensor.matmul(
                                op[:],
                                ctxT_v[:, cc, 128 * si:128 * si + 128],
                                w_o_sb[:, DIM * cc + 512 * u:DIM * cc + 512 * u + 512],
                                start=(cc == 0), stop=(cc == 3))
                        nc.vector.tensor_copy(ob[:, 512 * u:512 * u + 512], op[:])
                        if u == 1:
                            nc.sync.dma_start(
                                out=out_d[128 * si:128 * si + 128, :], in_=ob[:])
                    return emit

                if to_bg:
                    bg.append(_half(0))
                    bg.append(_half(1))
                elif to_pending:
                    pending.append(_half(0))
                    pending.append(_half(1))
                else:
                    _half(0)()
                    _half(1)()

            def tail_partial(ops, si):
                """out_proj cc=0..2 partial for tokens [128si, +128) -> SBUF.

                Runs inside the last unit's slots (hp<=2 ctxT ready); only
                the cc=3 matmul + add remains after the final transposes.
                """
                pt = obp.tile([128, DIM], mmdt, tag="pt", bufs=4,
                              name=f"pt_{si}")

                def _half(u):
                    def emit():
                        op = ops.tile([128, 512], F32, tag="pj", bufs=2,
                                      name=f"ptp_{si}_{u}")
                        for cc in range(3):
                            nc.tensor.matmul(
                                op[:],
                                ctxT_v[:, cc, 128 * si:128 * si + 128],
                                w_o_sb[:, DIM * cc + 512 * u:DIM * cc + 512 * u + 512],
                                start=(cc == 0), stop=(cc == 2))
                        nc.vector.tensor_copy(pt[:, 512 * u:512 * u + 512],
                                              op[:])
                    return emit
                bg.append(_half(0))
                bg.append(_half(1))
                return pt

            def tail_finish(ops, si, pt):
                """out_proj cc=3 + partial add + store, via pending."""
                ob = obp.tile([128, DIM], F32, tag="ob", name=f"ob_{si}")

                def _half(u):
                    def emit():
                        op = ops.tile([128, 512], F32, tag="pj", bufs=2,
                                      name=f"tf_{si}_{u}")
                        nc.tensor.matmul(
                            op[:],
                            ctxT_v[:, 3, 128 * si:128 * si + 128],
                            w_o_sb[:, 3 * DIM + 512 * u:3 * DIM + 512 * u + 512],
                            start=True, stop=True)
                        nc.vector.tensor_tensor(
                            ob[:, 512 * u:512 * u + 512], op[:],
                            pt[:, 512 * u:512 * u + 512],
                            op=mybir.AluOpType.add)
                        if u == 1:
                            nc.sync.dma_start(
                                out=out_d[128 * si:128 * si + 128, :],
                                in_=ob[:])
                    return emit
                pending.append(_half(0))
                pending.append(_half(1))

            # ---- emission schedule -------------------------------------
            # Startup DMAs ordered by first use on the exp critical path
            # (kv_lat -> q_lat -> KT/QT -> first QK); the cost model
            # serializes all DMA transfers on one shared device, so queue
            # order IS arrival order.
            with tc.tile_pool(name="pjps", bufs=1, space="PSUM") as pj:
                x_chunk(0, warm=True)
                x_chunk(1, warm=True)
                nc.sync.dma_start(out=w_kvc_sb[:], in_=w_kvc_d[:, :])
                x_chunk(2, warm=True)
                x_chunk(3, warm=True)
                nc.sync.dma_start(out=w_qc_sb[:], in_=w_qc_d[:, :])
                x_chunk(4)
                x_chunk(5)
                nc.sync.dma_start(out=w_kvu_k_sb[:], in_=w_kvu_k_d[:, :])
                nc.sync.dma_start(out=w_qu_sb[:], in_=w_qu_d[:, :])
                nc.sync.dma_start(out=b_all_sb[:], in_=b_all_d[:, :])
                nc.sync.dma_start(out=w_kvu_v_sb[:], in_=w_kvu_v_d[:, :])
                x_chunk(6)
                x_chunk(7)
                piece(pj, 0, 512, es=True)
                # tokens 512-767 projected inline in the pre-exp window
                # (ScalarE epilogues); tokens 768-1023 fill s-block 0's
                # later slots (x6/x7 landed by then)
                piece(pj, 512, 256, es=True)
                piece(pj, 768, 256, to_bg=True)
                for hp in range(4):
                    attn_unit(hp, 0, bg_n=(0, 2, 2, 2)[hp])
                x_stage(2)
                piece(pj, 1024, 512, to_bg=True)
                for hp in range(4):
                    attn_unit(hp, 512)
                x_stage(3)
                drain_bg(len(bg))
                wload_o()
            with tc.tile_pool(name="ops", bufs=1, space="PSUM") as ops:
                piece(ops, 1536, 512, to_bg=True)
                for hp in range(4):
                    attn_unit(hp, 1024)
                    out_chunk(ops, hp, to_bg=True)
                attn_unit(0, 1536)
                for si in range(4, 8):
                    out_chunk(ops, si, to_bg=True)
                attn_unit(1, 1536)
                for si in range(8, 12):
                    out_chunk(ops, si, to_bg=True)
                attn_unit(2, 1536)
                pts = {si: tail_partial(ops, si) for si in range(12, NT)}
                attn_unit(3, 1536, last=True, bg_delay=5,
                          on_chunk=lambda c: tail_finish(ops, 12 + c,
                                                         pts[12 + c]))
                drain(len(pending))
                drain_bg(len(bg))

    nc.finalize()
    return nc


def shard_inputs(inputs, S=2048):
    """Build the 8 per-core input maps (host-rounded, SBUF-layout)."""
    np_bf16 = mybir.dt.np(BF16)
    np_fp8 = mybir.dt.np(FP8)
    f = lambda a: np.ascontiguousarray(np.asarray(a, dtype=np.float32))

    def sb_layout(w, dt):
        """[c*128, q] -> [128, c*q] (partition-major chunks), cast to dt."""
        c = w.shape[0] // 128
        return np.ascontiguousarray(
            w.reshape(c, 128, -1).transpose(1, 0, 2).reshape(128, -1)
        ).astype(dt)

    x = f(inputs["x"])
    w_kvc, b_kvc = f(inputs["w_kvc"]), f(inputs["b_kvc"])
    w_kvu, b_kvu = f(inputs["w_kvu"]), f(inputs["b_kvu"])
    w_qc, b_qc = f(inputs["w_qc"]), f(inputs["b_qc"])
    w_qu, b_qu = f(inputs["w_qu"]), f(inputs["b_qu"])
    w_o = f(inputs["w_o"])
    w_kvc_l = sb_layout(w_kvc, np_bf16)
    w_qc_l = sb_layout(w_qc * WU_SCALE, np_fp8)
    # fold the latent biases into the up-projection biases
    b_qu_eff = b_qc @ w_qu + b_qu          # [1024]
    b_k_eff = b_kvc @ w_kvu[:, :1024] + b_kvu[:1024]
    in_maps = []
    for core in range(NCORES):
        b = core // 2
        g = core % 2
        cs = slice(512 * g, 512 * g + 512)
        in_maps.append({
            "x": x[b].astype(np_bf16),
            "w_kvc": w_kvc_l,
            "w_qc": w_qc_l,
            "w_kvu_k": w_kvu[:, 512 * g:512 * g + 512].astype(np_bf16),
            "w_kvu_v": w_kvu[:, 1024 + 512 * g:1024 + 512 * g + 512].astype(np_bf16),
            "w_qu": sb_layout(w_qu[:, cs] * WU_SCALE, np_fp8),
            "w_o": sb_layout(w_o[cs, :], np_bf16),
            "b_all": np.ascontiguousarray(np.concatenate([
                b_qu_eff[cs].reshape(4, 128).T,
                b_k_eff[cs].reshape(4, 128).T,
            ], axis=1)),
        })
    return in_maps


def gather_out(results, inputs, S=2048):
    """Sum the two per-batch partials and add the constant bias row."""
    f = lambda a: np.asarray(a, dtype=np.float32)
    # effective V bias incl. the folded b_kvc contribution
    b_v = f(inputs["b_kvc"]) @ f(inputs["w_kvu"])[:, DIM:] + f(inputs["b_kvu"])[DIM:]
    const_row = b_v @ f(inputs["w_o"]) + f(inputs["b_o"])
    out = np.empty((B, S, DIM), dtype=np.float32)
    for b in range(B):
        out[b] = results[2 * b]["out"] + results[2 * b + 1]["out"] + const_row
    return out


def kernel(**inputs) -> np.ndarray:
    from concourse.bass_utils import run_bass_kernel_spmd

    x = np.asarray(inputs["x"])
    S = x.shape[1]
    nc = build_mla(S=S)
    in_maps = shard_inputs(inputs, S=S)
    res = run_bass_kernel_spmd(nc, in_maps, list(range(NCORES))).results
    return gather_out(res, inputs, S=S)



# revision 57
# speedup vs baseline: 1.0150x; 1.0121x over previous
"""MLA (multi-head latent attention) Bass kernel for Trainium2, 8 NeuronCores.

Sharding: core i handles batch b = i // 2 and head-group g = i % 2
(8 of the 16 heads).  Each core computes a partial output (its heads'
contribution through out_proj); the host sums the two partials per batch
and adds a constant row (b_kvu_v @ w_o + b_o), which is exact because
softmax rows sum to 1 so the V-bias passes through attention additively.

All matmul operands are bf16 (1 cycle/row on the PE regardless of
output width); accumulation stays f32 in PSUM.  No PE transposes: both
x -> xT and ctx -> ctxT go through the DMA XBAR (dma_start_transpose,
2-byte dtypes) after an f32->bf16 rounding copy on GpSimd/DVE.

Structure (single TileContext; the Tile list-scheduler dispatches ready
instructions by emission-order priority, so emission IS the schedule):
  piece(p), p=0..3 (512 tokens each): latents kv_latT/q_latT{0,1}
    [128,S] = W^T xT (+bias, DVE); KT/QT [128, 4 chunks * S];
    V [128, NT*520] (64 cols/head + a ones col for the softmax denom).
  attention unit = (head pair hp, 512 queries): per key-chunk k one
    merged scores tile [128, 2 x fd] (both heads, disjoint 64-row PE
    groups) and ONE exp on ScalarE over a strided [128, 2, fd] AP
    (halves ScalarE's fixed per-call cost); causal diagonal via
    affine_select on Pool; PV re-uses exp tiles as stationary:
    ctx_psum[s-chunk] [128 queries, 65] accumulates over k, each
    s-chunk as one contiguous accumulation group (PSUM banks allow
    only one open group); per-partition reciprocal + scalar multiply
    (DVE) normalize into a token-major pair tile, DMA-transposed into
    ctxT.  Trailing PV groups/retires are deferred into the NEXT
    unit's slots ("pending") and projection pieces 2/3 + out_proj
    chunks are drained one sub-step per slot ("bg"), because the
    4-deep PE wait queue blocks later ready instructions behind
    waiting ones - bursts would starve ScalarE.
  out_proj per 128-token chunk: 4x128-contraction accumulate into
  [128,512] PSUM halves, DVE copy, DMA out; the last 4 chunks are
  gated per-128-column ctxT transposes of the final unit.
PSUM budget: scores 2x2 banks + ctx 2x1 + piece/out_proj ring 2 = 8.
"""

import numpy as np

import concourse.bass as bass
import concourse.bacc as bacc
import concourse.mybir as mybir
import concourse.tile as tile

DIM = 1024
NUM_HEADS = 16
HEAD_DIM = 64
LAT = 128
QR = 256
B = 4
NCORES = 8
ND = DIM // 128       # 8 d-chunks
NHL = 8               # heads per core
F32 = mybir.dt.float32
BF16 = mybir.dt.bfloat16
FP8 = mybir.dt.float8e4
AF = mybir.ActivationFunctionType
DR = mybir.MatmulPerfMode.DoubleRow

# fp8 is used ONLY on the Q-path (q_lat and QT projections): softmax squashes
# the ~2% fp8 rms error to ~0.1% on probs (scores err * 0.125 * score scale).
# The V/ctx/out path keeps bf16 -- fp8 there passes its full error to the
# output.  w_qc/w_qu are host-scaled by 64 into fp8-normal range; the
# projection epilogues descale by 1/64.
WU_SCALE = 64.0


def _pieces(total, w=512):
    return [(o, min(w, total - o)) for o in range(0, total, w)]


def build_mla(S=2048, mmdt=BF16):
    """Build the per-core Bass program (same SPMD program on all 8 cores)."""
    assert S % 512 == 0
    SH = S // 2           # s-half width
    NT = S // 128         # number of 128-token chunks
    NP = S // 512         # number of 512-token pieces

    nc = bacc.Bacc()

    # x and weights arrive host-rounded (bf16 / scaled fp8) in SBUF layout:
    # halves DMA bytes and removes all on-device staging/rounding copies.
    x_d = nc.declare_dram_parameter("x", [S, DIM], BF16, isOutput=False)
    w_kvc_d = nc.declare_dram_parameter("w_kvc", [128, ND * LAT], BF16,
                                        isOutput=False)
    w_qc_d = nc.declare_dram_parameter("w_qc", [128, ND * QR], FP8,
                                       isOutput=False)
    w_kvu_k_d = nc.declare_dram_parameter("w_kvu_k", [128, 512], BF16,
                                          isOutput=False)
    w_kvu_v_d = nc.declare_dram_parameter("w_kvu_v", [128, 512], BF16,
                                          isOutput=False)
    w_qu_d = nc.declare_dram_parameter("w_qu", [128, 1024], FP8,
                                       isOutput=False)
    w_o_d = nc.declare_dram_parameter("w_o", [128, 4 * DIM], BF16,
                                      isOutput=False)
    # b_kvc / b_qc are folded on the host into effective K/Q up-proj biases
    # (and the V-bias into the host const row), so the latent epilogues are
    # pure copies/scales: b_all = [b_qu_eff (4 chunks) | b_k_eff (4 chunks)]
    b_all_d = nc.declare_dram_parameter("b_all", [128, 8], F32, isOutput=False)
    # output leaves as bf16 (halves the out DMA bytes); host upcasts
    out_d = nc.declare_dram_parameter("out", [S, DIM], BF16, isOutput=True)

    with tile.TileContext(nc) as tc:
        with (
            tc.tile_pool(name="wts", bufs=1) as wts,
            tc.tile_pool(name="big", bufs=1) as big,
            tc.tile_pool(name="xbp", bufs=3) as xbp,
            tc.tile_pool(name="attn", bufs=1) as attn,
            tc.tile_pool(name="cpp", bufs=2) as cpp,
            tc.tile_pool(name="obp", bufs=2) as obp,
            tc.tile_pool(name="scps", bufs=1, space="PSUM") as scps,
            tc.tile_pool(name="ctxps", bufs=1, space="PSUM") as ctxps,
        ):
            # ---- persistent products -----------------------------------
            # xT is split per 512-token piece: the dependency tracker is
            # conservative across one big tile, so a single xT would make
            # later transposes wait on earlier pieces' matmul reads.
            xTp = [big.tile([128, ND * 512], mmdt, name=f"xT{j}")
                   for j in range(NP)]
            xTp_v = [t[:].rearrange("p (d t) -> p d t", d=ND) for t in xTp]
            # fp8 shadow of xT for the q_lat DoubleRow projection
            xT8p = [big.tile([128, ND * 512], FP8, name=f"xT8{j}")
                    for j in range(NP)]
            xT8p_v = [t[:].rearrange("p (d t) -> p d t", d=ND) for t in xT8p]
            kv_latT = big.tile([128, S], mmdt, name="kv_latT")
            # q_latT halves adjacent in one fp8 tile so QT can contract both
            # 128-blocks of QR in a single DoubleRow matmul
            q_latT = big.tile([128, 2 * S], FP8, name="q_latT")
            q_latT_v = q_latT[:].rearrange("p (g t) -> p g t", g=2)
            KT = big.tile([128, 4 * S], mmdt, name="KT")
            QT = big.tile([128, 4 * S], mmdt, name="QT")
            V = big.tile([128, NT * 520], mmdt, name="V")
            v_view = V[:].rearrange("p (k h c) -> p k h c", h=NHL, c=65)
            ctxT = big.tile([128, 4 * S], mmdt, name="ctxT")
            ctxT_v = ctxT[:].rearrange("p (c t) -> p c t", c=4)

            # ones columns of V (col 64 of each 65-wide head block)
            nc.gpsimd.memset(v_view[:, :, :, 64:65], 1.0)

            # ---- weights into SBUF (direct DMA, host-rounded) ----------
            w_kvc_sb = wts.tile([128, DIM], mmdt, name="w_kvc_sb")
            w_qc_sb = wts.tile([128, ND * QR], FP8, name="w_qc_sb")
            w_qc8_v = w_qc_sb[:].rearrange("p (d q) -> p d q", d=ND)
            w_kvu_k_sb = wts.tile([128, 512], mmdt, name="w_kvu_k_sb")
            w_kvu_v_sb = wts.tile([128, 512], mmdt, name="w_kvu_v_sb")
            w_qu_sb = wts.tile([128, 1024], FP8, name="w_qu_sb")
            w_qu8_v = w_qu_sb[:].rearrange("p (g q) -> p g q", g=2)
            w_o_sb = wts.tile([128, 4 * DIM], mmdt, name="w_o_sb")
            b_all_sb = wts.tile([128, 8], F32, name="b_all_sb")
            b_qu_sb = b_all_sb[:, 0:4]
            b_kvu_k_sb = b_all_sb[:, 4:8]

            def wload_o():
                # four DMAs, not one: a single 3.2us transfer head-of-line
                # blocks the shared DMA device at awkward times
                for cc in range(4):
                    nc.sync.dma_start(
                        out=w_o_sb[:, DIM * cc:DIM * cc + DIM],
                        in_=w_o_d[:, DIM * cc:DIM * cc + DIM])

            # ---- emission helpers --------------------------------------
            def x_chunk(q, warm=False):
                j, lq = q // 4, q % 4
                xb = xbp.tile([128, DIM], mmdt, tag="xb", bufs=3)
                nc.sync.dma_start(
                    out=xb[:], in_=x_d[128 * q:128 * q + 128, :])
                nc.sync.dma_start_transpose(
                    xTp_v[j][:, :, 128 * lq:128 * lq + 128], xb[:])
                nc.gpsimd.tensor_copy(
                    xT8p_v[j][:, :, 128 * lq:128 * lq + 128],
                    xTp_v[j][:, :, 128 * lq:128 * lq + 128])
                if warm:
                    # tiny matmuls keep the PE p-state ramp alive before the
                    # first projection burst
                    wps = scps.tile([128, 1024], F32, tag="sc", bufs=2,
                                    name=f"warm_{q}")
                    for i in range(8):
                        nc.tensor.matmul(wps[0:1, 0:1], xb[0:1, 0:1],
                                         xb[0:1, 0:1], start=True, stop=True)

            def x_stage(p):
                """Load, round, and DMA-transpose x tokens [512p, 512p+512)."""
                for q in range(4 * p, 4 * p + 4):
                    x_chunk(q)

            def piece(pj, o, width=512, to_bg=False, es=False):
                """All projections for tokens [o, o+width).

                With to_bg=True the sub-steps are queued on `bg` and drained
                one per attention slot, so they fill engine-idle time instead
                of preempting the next unit's QK matmuls.
                """
                p = o
                items = []

                xj, xo = o // 512, o % 512

                es_state = {"i": 0}

                def lat_epi(dst_ap, acc_ap, scale):
                    """Latent epilogue: pure copy/scale (biases host-folded).
                    es=True alternates ScalarE/DVE in the pre-exp startup
                    window so the two q-half copies run in parallel."""
                    es_state["i"] ^= 1
                    if es and es_state["i"]:
                        nc.scalar.activation(dst_ap, acc_ap, AF.Copy,
                                             scale=scale)
                    elif scale == 1.0:
                        nc.vector.tensor_copy(dst_ap, acc_ap)
                    else:
                        nc.vector.tensor_scalar_mul(dst_ap, acc_ap, scale)

                def _lat_kv():
                    state = {}

                    def part(d0, d1):
                        def emit():
                            if 'acc' not in state:
                                state['acc'] = pj.tile(
                                    [128, 512], F32, tag="pj", bufs=2,
                                    name=f"pjkv_{p}")
                            acc = state['acc']
                            for dc in range(d0, d1):
                                nc.tensor.matmul(
                                    acc[:, :width],
                                    w_kvc_sb[:, 128 * dc:128 * dc + 128],
                                    xTp_v[xj][:, dc, xo:xo + width],
                                    start=(dc == 0), stop=(dc == ND - 1))
                            if d1 == ND:
                                lat_epi(kv_latT[:, o:o + width],
                                        acc[:, :width], 1.0)
                        return emit
                    return [part(0, 3), part(3, 6), part(6, 8)]

                def _lat_q(coloff, dsto):
                    """q_lat half: DoubleRow fp8 over d-chunk pairs."""
                    state = {}

                    def part(p0, p1):
                        def emit():
                            if 'acc' not in state:
                                state['acc'] = pj.tile(
                                    [128, 512], F32, tag="pj", bufs=2,
                                    name=f"pjq_{p}_{coloff}")
                            acc = state['acc']
                            for dp in range(p0, p1):
                                nc.tensor.matmul(
                                    acc[:, :width],
                                    w_qc8_v[:, 2 * dp:2 * dp + 2,
                                            coloff:coloff + 128],
                                    xT8p_v[xj][:, 2 * dp:2 * dp + 2,
                                               xo:xo + width],
                                    start=(dp == 0), stop=(dp == ND // 2 - 1),
                                    perf_mode=DR)
                            if p1 == ND // 2:
                                lat_epi(q_latT[:, dsto + o:dsto + o + width],
                                        acc[:, :width], 1.0 / WU_SCALE)
                        return emit
                    return [part(0, 2), part(2, 4)]

                items.extend(_lat_kv())
                items.extend(_lat_q(0, 0))
                items.extend(_lat_q(128, S))

                def _qt(c):
                    def emit():
                        qp2 = pj.tile([128, 512], F32, tag="pj", bufs=2,
                                      name=f"pjq_{p}_{c}")
                        nc.tensor.matmul(
                            qp2[:, :width],
                            w_qu8_v[:, :, 128 * c:128 * c + 128],
                            q_latT_v[:, :, o:o + width],
                            start=True, stop=True, perf_mode=DR)
                        nc.vector.tensor_scalar(
                            QT[:, c * S + o:c * S + o + width], qp2[:, :width],
                            1.0 / WU_SCALE, b_qu_sb[:, c:c + 1],
                            op0=mybir.AluOpType.mult, op1=mybir.AluOpType.add)
                    return emit

                def _kt(c):
                    def emit():
                        kp = pj.tile([128, 512], F32, tag="pj", bufs=2,
                                     name=f"pjk_{p}_{c}")
                        nc.tensor.matmul(
                            kp[:, :width], w_kvu_k_sb[:, 128 * c:128 * c + 128],
                            kv_latT[:, o:o + width], start=True, stop=True)
                        nc.vector.tensor_scalar_add(
                            KT[:, c * S + o:c * S + o + width], kp[:, :width],
                            b_kvu_k_sb[:, c:c + 1])
                    return emit

                for c in range(4):
                    items.append(_qt(c))
                    items.append(_kt(c))

                def _v(q):
                    def emit():
                        vp = pj.tile([128, 512], F32, tag="pj", bufs=2,
                                     name=f"pjv_{q}")
                        nc.tensor.matmul(vp[:], kv_latT[:, 128 * q:128 * q + 128],
                                         w_kvu_v_sb[:], start=True, stop=True)
                        nc.vector.tensor_copy(
                            v_view[:, q, :, 0:64],
                            vp[:].rearrange("p (h c) -> p h c", c=64))
                    return emit

                for q in range(o // 128, (o + width) // 128):
                    items.append(_v(q))
                if to_bg:
                    bg.extend(items)
                else:
                    for it in items:
                        it()

            pending = []  # deferred closures, drained into later QK/exp slots
            bg = []       # background closures (pieces, out_proj), 1 per slot
            fin = []      # final out_proj finishes, emitted after the last unit

            def drain(n):
                for _ in range(min(n, len(pending))):
                    pending.pop(0)()

            def drain_bg(n):
                for _ in range(min(n, len(bg))):
                    bg.pop(0)()

            def attn_unit(hp, s0, SW=512, last=False, bg_n=1, bg_delay=0,
                          on_chunk=None):
                """Attention for queries [s0, s0+SW), both heads of pair hp.

                One merged exp per key-chunk covers both heads ([128, 2, fd]
                strided AP over a shared scores tile).  PV accumulation
                groups are emitted contiguously per (head, s-chunk) -- PSUM
                banks support only one open group at a time -- lagged into
                the QK/exp slots; trailing groups, the retires, and the
                ctxT transposes are deferred into the NEXT unit's slots via
                `pending` (the 4-deep PE wait queue would otherwise block
                the next unit's QK behind not-yet-ready PV matmuls).
                """
                base = s0 // 128
                nch = SW // 128
                kmax = base + nch
                cp = cpp.tile([128, 512], mmdt, tag="cp", name=f"cp_{hp}_{s0}")
                ctxs = [ctxps.tile([128, 512], F32, tag="ctx", bufs=2,
                                   name=f"ctx_{hp}_{s0}_{i}")
                        for i in range(2)]
                exs = []

                def pv_group(h2, c):
                    h = 2 * hp + h2
                    ctx = ctxs[h2]
                    klast = base + c

                    def emit():
                        for k in range(klast + 1):
                            rel = max(s0, 128 * k) - s0
                            cs = 512 * h2 + 128 * c - rel
                            nc.tensor.matmul(
                                ctx[:, 128 * c:128 * c + 65],
                                exs[k][:, cs:cs + 128],
                                V[:, 520 * k + 65 * h:520 * k + 65 * h + 65],
                                start=(k == 0), stop=(k == klast))
                        if last:
                            rec = attn.tile([128, 1], F32, tag="rec1", bufs=4,
                                            name=f"rec1_{hp}_{s0}_{h2}_{c}")
                            nc.vector.reciprocal(
                                rec[:], ctx[:, 128 * c + 64:128 * c + 65])
                            nc.vector.tensor_scalar_mul(
                                cp[:, 128 * c + 64 * h2:128 * c + 64 * h2 + 64],
                                ctx[:, 128 * c:128 * c + 64], rec[:, 0:1])
                            if h2 == 1:
                                nc.sync.dma_start_transpose(
                                    ctxT_v[:, hp, s0 + 128 * c:s0 + 128 * c + 128]
                                    .rearrange("p (b t) -> p b t", t=128),
                                    cp[:, 128 * c:128 * c + 128])
                                if on_chunk is not None:
                                    on_chunk(c)
                    return emit

                def retire(h2):
                    ctx = ctxs[h2]

                    def emit():
                        rec = attn.tile([128, 4], F32, tag="rec", bufs=4,
                                        name=f"rec_{hp}_{s0}_{h2}")
                        nc.vector.reciprocal(
                            rec[:, :nch],
                            ctx[:].rearrange("p (c u) -> p c u", u=128)[:, :nch, 64])
                        for c in range(nch):
                            nc.vector.tensor_scalar_mul(
                                cp[:, 128 * c + 64 * h2:128 * c + 64 * h2 + 64],
                                ctx[:, 128 * c:128 * c + 64], rec[:, c:c + 1])
                    return emit

                def tp():
                    def emit():
                        nc.sync.dma_start_transpose(
                            ctxT_v[:, hp, s0:s0 + SW].rearrange(
                                "p (b t) -> p b t", t=128),
                            cp[:, :SW])
                    return emit

                lag = 2 if s0 == 0 else 0
                inslot = nch if last else max(1, nch - 2)
                emitted = 0
                for k in range(kmax):
                    t0 = 128 * k
                    ss = max(s0, t0)
                    fd = s0 + SW - ss
                    sc = scps.tile([128, 1024], F32, tag="sc", bufs=2,
                                   name=f"sc_{hp}_{s0}_{k}")
                    for h2 in range(2):
                        po = 64 * h2
                        nc.tensor.matmul(
                            sc[:, 512 * h2:512 * h2 + fd],
                            KT[po:po + 64, hp * S + t0:hp * S + t0 + 128],
                            QT[po:po + 64, hp * S + ss:hp * S + ss + fd],
                            start=True, stop=True)
                    ex = attn.tile([128, 1024], mmdt, tag="ex", bufs=21,
                                   name=f"ex_{hp}_{s0}_{k}")
                    exs.append(ex)
                    sc3 = sc[:].rearrange("p (g q) -> p g q", g=2)[:, :, :fd]
                    ex3 = ex[:].rearrange("p (g q) -> p g q", g=2)[:, :, :fd]
                    nc.scalar.activation(ex3, sc3, AF.Exp, scale=0.125)
                    if t0 >= s0:
                        nc.gpsimd.affine_select(
                            out=ex[:].rearrange("p (g q) -> p g q", g=2)[:, :, 0:128],
                            in_=ex[:].rearrange("p (g q) -> p g q", g=2)[:, :, 0:128],
                            pattern=[[0, 2], [1, 128]],
                            compare_op=mybir.AluOpType.is_ge,
                            fill=0.0, base=0, channel_multiplier=-1)
                    drain(2)
                    if k >= bg_delay:
                        drain_bg(bg_n)
                    c = k - base - lag
                    if 0 <= c < inslot:
                        pv_group(0, c)()
                        pv_group(1, c)()
                        emitted = c + 1
                for c in range(emitted, nch):
                    if last or nch <= 2:
                        pv_group(0, c)()
                        pv_group(1, c)()
                    else:
                        pending.append(pv_group(0, c))
                        pending.append(pv_group(1, c))
                if not last:
                    pending.append(retire(0))
                    pending.append(retire(1))
                    pending.append(tp())

            def out_chunk(ops, si, to_bg=False, to_pending=False):
                """out_proj for tokens [128si, 128si+128)."""
                ob = obp.tile([128, DIM], mmdt, tag="ob", name=f"ob_{si}")

                def _half(u):
                    def emit():
                        op = ops.tile([128, 512], F32, tag="pj", bufs=2,
                                      name=f"op_{si}_{u}")
                        for cc in range(4):
                            nc.tensor.matmul(
                                op[:],
                                ctxT_v[:, cc, 128 * si:128 * si + 128],
                                w_o_sb[:, DIM * cc + 512 * u:DIM * cc + 512 * u + 512],
                                start=(cc == 0), stop=(cc == 3))
                        nc.vector.tensor_copy(ob[:, 512 * u:512 * u + 512], op[:])
                        if u == 1:
                            nc.sync.dma_start(
                                out=out_d[128 * si:128 * si + 128, :], in_=ob[:])
                    return emit

                if to_bg:
                    bg.append(_half(0))
                    bg.append(_half(1))
                elif to_pending:
                    pending.append(_half(0))
                    pending.append(_half(1))
                else:
                    _half(0)()
                    _half(1)()

            def tail_partial(ops, si):
                """out_proj cc=0..2 partial for tokens [128si, +128) -> SBUF.

                Runs inside the last unit's slots (hp<=2 ctxT ready); only
                the cc=3 matmul + add remains after the final transposes.
                """
                pt = obp.tile([128, DIM], mmdt, tag="pt", bufs=4,
                              name=f"pt_{si}")

                def _half(u):
                    def emit():
                        op = ops.tile([128, 512], F32, tag="pj", bufs=2,
                                      name=f"ptp_{si}_{u}")
                        for cc in range(3):
                            nc.tensor.matmul(
                                op[:],
                                ctxT_v[:, cc, 128 * si:128 * si + 128],
                                w_o_sb[:, DIM * cc + 512 * u:DIM * cc + 512 * u + 512],
                                start=(cc == 0), stop=(cc == 2))
                        nc.vector.tensor_copy(pt[:, 512 * u:512 * u + 512],
                                              op[:])
                    return emit
                bg.append(_half(0))
                bg.append(_half(1))
                return pt

            def tail_finish(ops, si, pt):
                """out_proj cc=3 + partial add + store, deferred to fin."""
                ob = obp.tile([128, DIM], mmdt, tag="ob", name=f"ob_{si}")

                def _half(u):
                    def emit():
                        op = ops.tile([128, 512], F32, tag="pj", bufs=2,
                                      name=f"tf_{si}_{u}")
                        nc.tensor.matmul(
                            op[:],
                            ctxT_v[:, 3, 128 * si:128 * si + 128],
                            w_o_sb[:, 3 * DIM + 512 * u:3 * DIM + 512 * u + 512],
                            start=True, stop=True)
                        nc.vector.tensor_tensor(
                            ob[:, 512 * u:512 * u + 512], op[:],
                            pt[:, 512 * u:512 * u + 512],
                            op=mybir.AluOpType.add)
                        if u == 1:
                            nc.sync.dma_start(
                                out=out_d[128 * si:128 * si + 128, :],
                                in_=ob[:])
                    return emit
                fin.append(_half(0))
                fin.append(_half(1))

            # ---- emission schedule -------------------------------------
            # Startup DMAs ordered by first use on the exp critical path
            # (kv_lat -> q_lat -> KT/QT -> first QK); the cost model
            # serializes all DMA transfers on one shared device, so queue
            # order IS arrival order.
            with tc.tile_pool(name="pjps", bufs=1, space="PSUM") as pj:
                x_chunk(0, warm=True)
                x_chunk(1, warm=True)
                nc.sync.dma_start(out=w_kvc_sb[:], in_=w_kvc_d[:, :])
                x_chunk(2, warm=True)
                x_chunk(3, warm=True)
                nc.sync.dma_start(out=w_qc_sb[:], in_=w_qc_d[:, :])
                x_chunk(4)
                x_chunk(5)
                nc.sync.dma_start(out=w_kvu_k_sb[:], in_=w_kvu_k_d[:, :])
                nc.sync.dma_start(out=w_qu_sb[:], in_=w_qu_d[:, :])
                nc.sync.dma_start(out=b_all_sb[:], in_=b_all_d[:, :])
                nc.sync.dma_start(out=w_kvu_v_sb[:], in_=w_kvu_v_d[:, :])
                x_chunk(6)
                x_chunk(7)
                piece(pj, 0, 512, es=True)
                # tokens 512-767 projected inline in the pre-exp window
                # (ScalarE epilogues); tokens 768-1023 fill s-block 0's
                # later slots (x6/x7 landed by then)
                piece(pj, 512, 256, es=True)
                piece(pj, 768, 256, to_bg=True)
                for hp in range(4):
                    attn_unit(hp, 0, bg_n=(0, 2, 2, 2)[hp])
                x_stage(2)
                piece(pj, 1024, 512, to_bg=True)
                for hp in range(4):
                    attn_unit(hp, 512)
                x_stage(3)
                drain_bg(len(bg))
                wload_o()
            with tc.tile_pool(name="ops", bufs=1, space="PSUM") as ops:
                piece(ops, 1536, 512, to_bg=True)
                for hp in range(4):
                    attn_unit(hp, 1024)
                    out_chunk(ops, hp, to_bg=True)
                attn_unit(0, 1536)
                for si in range(4, 8):
                    out_chunk(ops, si, to_bg=True)
                attn_unit(1, 1536)
                for si in range(8, 12):
                    out_chunk(ops, si, to_bg=True)
                attn_unit(2, 1536)
                pts = {si: tail_partial(ops, si) for si in range(12, NT)}
                attn_unit(3, 1536, last=True, bg_delay=5,
                          on_chunk=lambda c: tail_finish(ops, 12 + c,
                                                         pts[12 + c]))
                for it in fin:
                    it()
                drain(len(pending))
                drain_bg(len(bg))

    nc.finalize()
    return nc


def shard_inputs(inputs, S=2048):
    """Build the 8 per-core input maps (host-rounded, SBUF-layout)."""
    np_bf16 = mybir.dt.np(BF16)
    np_fp8 = mybir.dt.np(FP8)
    f = lambda a: np.ascontiguousarray(np.asarray(a, dtype=np.float32))

    def sb_layout(w, dt):
        """[c*128, q] -> [128, c*q] (partition-major chunks), cast to dt."""
        c = w.shape[0] // 128
        return np.ascontiguousarray(
            w.reshape(c, 128, -1).transpose(1, 0, 2).reshape(128, -1)
        ).astype(dt)

    x = f(inputs["x"])
    w_kvc, b_kvc = f(inputs["w_kvc"]), f(inputs["b_kvc"])
    w_kvu, b_kvu = f(inputs["w_kvu"]), f(inputs["b_kvu"])
    w_qc, b_qc = f(inputs["w_qc"]), f(inputs["b_qc"])
    w_qu, b_qu = f(inputs["w_qu"]), f(inputs["b_qu"])
    w_o = f(inputs["w_o"])
    w_kvc_l = sb_layout(w_kvc, np_bf16)
    w_qc_l = sb_layout(w_qc * WU_SCALE, np_fp8)
    # fold the latent biases into the up-projection biases
    b_qu_eff = b_qc @ w_qu + b_qu          # [1024]
    b_k_eff = b_kvc @ w_kvu[:, :1024] + b_kvu[:1024]
    in_maps = []
    for core in range(NCORES):
        b = core // 2
        g = core % 2
        cs = slice(512 * g, 512 * g + 512)
        in_maps.append({
            "x": x[b].astype(np_bf16),
            "w_kvc": w_kvc_l,
            "w_qc": w_qc_l,
            "w_kvu_k": w_kvu[:, 512 * g:512 * g + 512].astype(np_bf16),
            "w_kvu_v": w_kvu[:, 1024 + 512 * g:1024 + 512 * g + 512].astype(np_bf16),
            "w_qu": sb_layout(w_qu[:, cs] * WU_SCALE, np_fp8),
            "w_o": sb_layout(w_o[cs, :], np_bf16),
            "b_all": np.ascontiguousarray(np.concatenate([
                b_qu_eff[cs].reshape(4, 128).T,
                b_k_eff[cs].reshape(4, 128).T,
            ], axis=1)),
        })
    return in_maps


def gather_out(results, inputs, S=2048):
    """Sum the two per-batch partials and add the constant bias row."""
    f = lambda a: np.asarray(a, dtype=np.float32)
    # effective V bias incl. the folded b_kvc contribution
    b_v = f(inputs["b_kvc"]) @ f(inputs["w_kvu"])[:, DIM:] + f(inputs["b_kvu"])[DIM:]
    const_row = b_v @ f(inputs["w_o"]) + f(inputs["b_o"])
    out = np.empty((B, S, DIM), dtype=np.float32)
    for b in range(B):
        out[b] = (np.asarray(results[2 * b]["out"], dtype=np.float32)
                  + np.asarray(results[2 * b + 1]["out"], dtype=np.float32)
                  + const_row)
    return out


def kernel(**inputs) -> np.ndarray:
    from concourse.bass_utils import run_bass_kernel_spmd

    x = np.asarray(inputs["x"])
    S = x.shape[1]
    nc = build_mla(S=S)
    in_maps = shard_inputs(inputs, S=S)
    res = run_bass_kernel_spmd(nc, in_maps, list(range(NCORES))).results
    return gather_out(res, inputs, S=S)

